# revision 41
# baseline (speedup 1.0000x reference)
"""Trainium2 Bass kernel for AdaptiveMHFConv (FNO-style spectral conv).

out = irfft2( pad_32x32( einsum('bhixy,hioxy', rfft2(x)[..., :32, :32], w) ) ) + bias

Sharding: 8 cores = 4 heads x 2 batch-halves. Each core handles 16 batches x
one head (16 in / 16 out channels) independently -- no collectives.

Per-core pipeline (all DFTs are truncated-DFT matmuls; modes = 32x32):
  S1  per image: lhsT=x[h,w] (stationary), rhs=EH[h,(kxr|kxi)] -> P1[w,64] psum
  S2  lhsT=FW[:,0:32]/[:,32:64] (ky re/im), rhs=P1s[w, 8img*64] -> A,B[32,512]
  C2  DVE combine -> CB[b][ky32, (kx32,i16,ri2)]
  T1  PE-transpose 128-chunks -> MTall[128=(kxl4,i16,ri2), b*256+(q8,ky32)]
  S3  modal: lhsT=WBD[q,ky][128,128] block-diag complex, rhs=MTall cols (b16)
      -> MO[q][(o16,kxl4,ri2), (ky,b)] -> reorder -> MOs[128, q*512+(b,ky)]
  S4  kx-inverse, accumulate over q: lhsT=IEQr/i[q][8,h'128],
      rhs=MOs[8o:8o+8, q-slice] -> Zr,Zi[h', (b,ky)] -> Zs[o]
  T3  per (o,b): PE-transpose Zs[128,(ri2,ky32)-cols] -> ZT[(ri2,ky32)=64, h']
  S5  lhsT=ZT, rhs=CW[(ri,ky), w'] -> y[h', w'] psum -> out DMA
"""
import os
import sys

import numpy as np

sys.path.insert(0, "/opt/trn_rl_repo")

import concourse.bass as bass  # noqa: E402
import concourse.mybir as mybir  # noqa: E402
from concourse.bass_utils import bass_rust, run_bass_kernel_spmd  # noqa: E402
from concourse.masks import make_identity  # noqa: E402
from concourse.tile import TileContext  # noqa: E402

F32 = mybir.dt.float32
F16 = mybir.dt.float16
NB = 16  # batches per core


def _build_shared_consts():
    h = np.arange(128)
    k32 = np.arange(32)
    ang = 2 * np.pi * np.outer(h, k32) / 128.0
    EH = np.concatenate([np.cos(ang), -np.sin(ang)], axis=1).astype(np.float32)
    FW3 = np.concatenate([np.cos(ang), np.sin(ang), -np.sin(ang)],
                         axis=1).astype(np.float32)
    # IEQR/IEQI [64, 128]: rows (kxl4, ri2, q8) matching the per-o slab
    # fold (8 rows x (q,b,ky) -> 64 rows x (b,ky)); cols h'.
    IEQR = np.zeros((64, 128), np.float32)
    IEQI = np.zeros((64, 128), np.float32)
    for kxl in range(4):
        for q in range(8):
            kx = q * 4 + kxl
            a = 2 * np.pi * h * kx / 128.0
            IEQR[kxl * 16 + 0 * 8 + q] = np.cos(a) / 128.0
            IEQR[kxl * 16 + 1 * 8 + q] = -np.sin(a) / 128.0
            IEQI[kxl * 16 + 0 * 8 + q] = np.sin(a) / 128.0
            IEQI[kxl * 16 + 1 * 8 + q] = np.cos(a) / 128.0
    CW = np.zeros((64, 128), np.float32)
    for ky in range(32):
        c = 1.0 if ky == 0 else 2.0
        a = 2 * np.pi * h * ky / 128.0
        CW[ky] = c * np.cos(a) / 128.0
        CW[32 + ky] = -c * np.sin(a) / 128.0
    CW[32] = 0.0  # irfft ignores Im of bin 0
    CW = np.concatenate([CW, CW], axis=0)  # [128, 128] both bases
    return EH, FW3, IEQR, IEQI, CW


def _build_wbd(w_real_h, w_imag_h):
    """[16i,16o,32kx,32ky] -> WBD [256, 128, 128] block-diag complex tiles."""
    Wr = np.ascontiguousarray(w_real_h.transpose(2, 3, 0, 1)).reshape(8, 4, 32, 16, 16)
    Wi = np.ascontiguousarray(w_imag_h.transpose(2, 3, 0, 1)).reshape(8, 4, 32, 16, 16)
    Wr = Wr.transpose(0, 2, 1, 3, 4)  # [q, ky, kxl, i, o]
    Wi = Wi.transpose(0, 2, 1, 3, 4)
    B7 = np.zeros((8, 32, 4, 16, 2, 16, 2), np.float32)  # [q,ky,kxl,i,ri,o,ri']
    B7[:, :, :, :, 0, :, 0] = Wr
    B7[:, :, :, :, 0, :, 1] = Wi
    B7[:, :, :, :, 1, :, 0] = -Wi
    B7[:, :, :, :, 1, :, 1] = Wr
    full = np.zeros((8, 32, 4, 16, 2, 16, 4, 2), np.float32)
    kk = np.arange(4)
    # rows (kxl,i,ri), cols (o,kxl,ri') -- diagonal in kxl
    full[:, :, kk, :, :, :, kk, :] = B7.transpose(2, 0, 1, 3, 4, 5, 6)
    return np.ascontiguousarray(full.reshape(256, 128, 128))


class _PhaseStop(Exception):
    pass


def _build_graph():
    nc = bass.Bass()
    x_ext = nc.declare_dram_parameter("x", [32, 128, 1024], F16, isOutput=False)
    eh_ext = nc.declare_dram_parameter("eh", [128, 64], F16, isOutput=False)
    fw3_ext = nc.declare_dram_parameter("fw3", [128, 96], F16, isOutput=False)
    wbd_ext = nc.declare_dram_parameter("wbd", [128, 32768], F16, isOutput=False)
    ieqr_ext = nc.declare_dram_parameter("ieqr", [64, 128], F16, isOutput=False)
    ieqi_ext = nc.declare_dram_parameter("ieqi", [64, 128], F16, isOutput=False)
    cw_ext = nc.declare_dram_parameter("cw", [128, 128], F16, isOutput=False)
    y_ext = nc.declare_dram_parameter("out", [16, 4, 128, 512], F16, isOutput=True)

    with TileContext(nc) as tc:
      try:
        with (
            tc.tile_pool(name="consts", bufs=1) as cpool,
            tc.tile_pool(name="slabs", bufs=1) as spool,
            tc.tile_pool(name="work", bufs=6) as wpool,
            tc.tile_pool(name="xts", bufs=16) as xpool,
        ):
            eh_sb = cpool.tile([128, 64], F16, tag="eh")
            nc.sync.dma_start(out=eh_sb, in_=eh_ext[:])
            fw3_sb = cpool.tile([128, 96], F16, tag="fw3")
            nc.sync.dma_start(out=fw3_sb, in_=fw3_ext[:])
            ieqr_sb = cpool.tile([64, 128], F16, tag="ieqr")
            nc.sync.dma_start(out=ieqr_sb, in_=ieqr_ext[:])
            ieqi_sb = cpool.tile([64, 128], F16, tag="ieqi")
            nc.sync.dma_start(out=ieqi_sb, in_=ieqi_ext[:])
            cw_sb = cpool.tile([128, 128], F16, tag="cw")
            nc.sync.dma_start(out=cw_sb, in_=cw_ext[:])
            ident = cpool.tile([128, 128], F16, tag="ident")
            make_identity(nc, ident)

            mtall = spool.tile([128, NB * 256], F16, tag="mtall")
            mosA = spool.tile([64, 4096], F16, tag="mosA")
            mosB = spool.tile([64, 4096], F16, tag="mosB")
            wslab = spool.tile([128, 32768], F16, tag="wslab")
            nc.sync.dma_start(out=wslab, in_=wbd_ext[:])

            # ---------------- Phase F: forward DFTs ----------------
            psF_cm = tc.tile_pool(name="psF", bufs=2, space="PSUM")
            psF = psF_cm.__enter__()
            psF1_cm = tc.tile_pool(name="psF1", bufs=3, space="PSUM")
            psF1 = psF1_cm.__enter__()
            psFt_cm = tc.tile_pool(name="psFt", bufs=1, space="PSUM")
            psFt = psFt_cm.__enter__()
            for b in range(NB):
                cb = wpool.tile([32, 1024], F16, tag="cb")
                for g in range(2):
                    # p1 cols: [img8*kx32 real | img8*kx32 imag]
                    p1 = psF1.tile([128, 512], F32, tag="p1")
                    xt8 = xpool.tile([128, 1024], F16, tag="xt8")
                    nc.sync.dma_start(out=xt8, in_=x_ext[b * 2 + g])
                    for j in range(8):
                        nc.tensor.matmul(p1[:, j * 64:(j + 1) * 64],
                                         lhsT=xt8[:, j * 128:(j + 1) * 128],
                                         rhs=eh_sb, start=True, stop=True)
                    # copy + de-interleave (img,ri,kx) -> (ri,img,kx)
                    p1s = wpool.tile([128, 512], F16, tag="p1s")
                    nc.scalar.copy(
                        out=p1s.rearrange("p (r j k) -> p j r k", r=2, j=8),
                        in_=p1.rearrange("p (j r k) -> p j r k", r=2, j=8))
                    # Xr = cos.P1r + sin.P1i ; Xi = cos.P1i - sin.P1r
                    xr = psF.tile([32, 256], F32, tag="xr")
                    xi = psF.tile([32, 256], F32, tag="xi")
                    nc.tensor.matmul(xr, lhsT=fw3_sb[:, 0:32],
                                     rhs=p1s[:, 0:256], start=True, stop=False)
                    nc.tensor.matmul(xr, lhsT=fw3_sb[:, 32:64],
                                     rhs=p1s[:, 256:512], start=False, stop=True)
                    nc.tensor.matmul(xi, lhsT=fw3_sb[:, 0:32],
                                     rhs=p1s[:, 256:512], start=True, stop=False)
                    nc.tensor.matmul(xi, lhsT=fw3_sb[:, 64:96],
                                     rhs=p1s[:, 0:256], start=False, stop=True)
                    # scatter into CB [32, (kx32, i16, ri2)]
                    cbv = cb.rearrange("p (kx i r) -> p i kx r", i=16, r=2)
                    i0 = g * 8
                    nc.vector.tensor_copy(
                        out=cbv[:, i0:i0 + 8, :, 0],
                        in_=xr.rearrange("p (j k) -> p j k", j=8))
                    nc.vector.tensor_copy(
                        out=cbv[:, i0:i0 + 8, :, 1],
                        in_=xi.rearrange("p (j k) -> p j k", j=8))
                # T1: 8 transposes [32,128] -> [128,32]
                pt = psFt.tile([128, 256], F16, tag="pt")
                for q in range(8):
                    nc.tensor.transpose(pt[:, q * 32:(q + 1) * 32],
                                        cb[:, q * 128:(q + 1) * 128],
                                        ident[0:32, 0:32])
                nc.vector.tensor_copy(out=mtall[:, b * 256:(b + 1) * 256], in_=pt)

            psFt_cm.__exit__(None, None, None)
            psF1_cm.__exit__(None, None, None)
            psF_cm.__exit__(None, None, None)

            if os.environ.get("KPHASES", "FMI") == "F":
                raise _PhaseStop()
            # ---------------- Phase M: modal contraction ----------------
            mtv = mtall.rearrange("p (b u) -> p u b", u=256)  # [128, 256, 16]
            wsv = wslab.rearrange("r (u c) -> r u c", c=128)
            with tc.tile_pool(name="psM", bufs=2, space="PSUM") as psM:
                for q in range(8):
                    mo = psM.tile([128, 512], F32, tag="mo")
                    for ky in range(32):
                        u = q * 32 + ky
                        nc.tensor.matmul(mo[:, ky * 16:ky * 16 + NB],
                                         lhsT=wsv[:, u, :],
                                         rhs=mtv[:, u, :], start=True, stop=True)
                    for rb, mosx in ((0, mosA), (64, mosB)):
                        nc.vector.tensor_copy(
                            out=mosx[:, q * 512:(q + 1) * 512].rearrange(
                                "p (b k) -> p k b", b=16),
                            in_=mo[rb:rb + 64, :].rearrange(
                                "p (k b) -> p k b", b=16))

            if os.environ.get("KPHASES", "FMI") == "FM":
                raise _PhaseStop()
            # ---------------- Phase I: inverse DFTs ----------------
            # matmul operands need partition base in {0,32,64}; DMA the 8-row
            # o-slices of mos down to base-0 tiles, 4 o's (one out-group) at
            # a time to bound SBUF.
            with (
                tc.tile_pool(name="zsp", bufs=2) as zspool,
                tc.tile_pool(name="slb", bufs=8) as slbpool,
                tc.tile_pool(name="psI", bufs=2, space="PSUM") as psI,
            ):
                for og in range(4):
                    zsg = zspool.tile([128, 4096], F16, tag="zs")
                    mosx = mosA if og < 2 else mosB
                    for ol in range(4):
                        o = og * 4 + ol
                        op = o % 8
                        # Fold [8, (q8,b,ky)] -> [64=(kxl,ri,q), (b,ky)]:
                        # same flat element order, repartitioned by DMA.
                        slab = slbpool.tile([64, 512], F16, tag="slab")
                        nc.sync.dma_start(
                            out=slab.rearrange("p (b k) -> p b k", b=16),
                            in_=mosx[op * 8:op * 8 + 8, :].rearrange(
                                "p (q b k) -> p q b k", q=8, b=16))
                        zr = psI.tile([128, 512], F32, tag="zr")
                        zi = psI.tile([128, 512], F32, tag="zi")
                        nc.tensor.matmul(zr, lhsT=ieqr_sb, rhs=slab,
                                         start=True, stop=True)
                        nc.tensor.matmul(zi, lhsT=ieqi_sb, rhs=slab,
                                         start=True, stop=True)
                        # zsg free layout: (o4, b16, ri2, ky32)
                        zsgv = zsg.rearrange("p (o b r k) -> p o b r k",
                                             o=4, b=16, r=2)
                        nc.scalar.copy(out=zsgv[:, ol, :, 0, :],
                                       in_=zr.rearrange("p (b k) -> p b k", b=16))
                        nc.scalar.copy(out=zsgv[:, ol, :, 1, :],
                                       in_=zi.rearrange("p (b k) -> p b k", b=16))
                    for b2 in range(NB // 2):
                        b0 = b2 * 2
                        zt_ps = psI.tile([128, 512], F16, tag="zt")
                        for ol in range(4):
                            # [128, (b2,(ri,ky))=128] -> [128=(b2,ri,ky), h']
                            nc.tensor.transpose(
                                zt_ps[:, ol * 128:(ol + 1) * 128],
                                zsg[:, ol * 1024 + b0 * 64:
                                    ol * 1024 + b0 * 64 + 128],
                                ident)
                        zt_sb = wpool.tile([128, 512], F16, tag="ztsb")
                        nc.vector.tensor_copy(out=zt_sb, in_=zt_ps)
                        for db in range(2):
                            b = b0 + db
                            rb = db * 64
                            # lhsT = CW (stationary const) -> y transposed
                            # [w', (ol, h')]; host unpermutes.
                            y_ps = psI.tile([128, 512], F32, tag="y")
                            for ol in range(4):
                                nc.tensor.matmul(
                                    y_ps[:, ol * 128:(ol + 1) * 128],
                                    lhsT=cw_sb[rb:rb + 64, :],
                                    rhs=zt_sb[rb:rb + 64,
                                              ol * 128:(ol + 1) * 128],
                                    start=True, stop=True)
                            out_sb = wpool.tile([128, 512], F16, tag="outsb")
                            if db == 0:
                                nc.scalar.copy(out=out_sb, in_=y_ps)
                            else:
                                nc.vector.tensor_copy(out=out_sb, in_=y_ps)
                            nc.sync.dma_start(out=y_ext[b, og], in_=out_sb)
      except _PhaseStop:
        pass
    # TRN2 instructions carry limited sync-wait slots; split excess waits
    # (same passes Bacc.compile runs).
    bass_rust.move_matmul_waits_to_ldweights(nc.m)
    bass_rust.generate_event_semaphores(nc)
    return nc


_CACHE = {}


def kernel(x, w_real, w_imag, bias):
    x = np.asarray(x, np.float32)
    w_real = np.asarray(w_real, np.float32)
    w_imag = np.asarray(w_imag, np.float32)
    bias = np.asarray(bias, np.float32)

    if "nc" not in _CACHE:
        _CACHE["nc"] = _build_graph()
        _CACHE["consts"] = _build_shared_consts()
    nc = _CACHE["nc"]
    EH, FW3, IEQR, IEQI, CW = _CACHE["consts"]

    in_maps = []
    for c in range(8):
        head, half = c // 2, c % 2
        xs = x[half * 16:(half + 1) * 16, head * 16:(head + 1) * 16]
        # [16b,16i,128,128] -> [(b,g), h, (i8, w)] image-interleaved
        xs = np.ascontiguousarray(
            xs.reshape(16, 2, 8, 128, 128).transpose(0, 1, 3, 2, 4)
        ).reshape(32, 128, 1024).astype(np.float16)
        key = ("wbd", head)
        if key not in _CACHE:
            wb = _build_wbd(w_real[head], w_imag[head])  # [256u,128r,128c]
            _CACHE[key] = np.ascontiguousarray(
                wb.transpose(1, 0, 2)).reshape(128, 32768).astype(np.float16)
        in_maps.append({"x": xs, "eh": EH.astype(np.float16),
                        "fw3": FW3.astype(np.float16),
                        "wbd": _CACHE[key], "ieqr": IEQR.astype(np.float16),
                        "ieqi": IEQI.astype(np.float16),
                        "cw": CW.astype(np.float16)})

    trace = os.environ.get("KERNEL_TRACE", "0") == "1"
    res = run_bass_kernel_spmd(nc, in_maps, core_ids=list(range(8)), trace=trace)
    _CACHE["exec_time_ns"] = res.exec_time_ns

    out = np.empty((32, 64, 128, 128), np.float32)
    for c in range(8):
        head, half = c // 2, c % 2
        # [b, og, w', (ol, h')] -> [b, o, h, w]
        ys = res.results[c]["out"].astype(np.float32).reshape(
            16, 4, 128, 4, 128).transpose(0, 1, 3, 4, 2).reshape(
            16, 16, 128, 128)
        out[half * 16:(half + 1) * 16, head * 16:(head + 1) * 16] = ys
    return out + bias[None]


# revision 42
# speedup vs baseline: 1.0102x; 1.0102x over previous
"""Trainium2 Bass kernel for AdaptiveMHFConv (FNO-style spectral conv).

out = irfft2( pad_32x32( einsum('bhixy,hioxy', rfft2(x)[..., :32, :32], w) ) ) + bias

Sharding: 8 cores = 4 heads x 2 batch-halves. Each core handles 16 batches x
one head (16 in / 16 out channels) independently -- no collectives.

Per-core pipeline (all DFTs are truncated-DFT matmuls; modes = 32x32):
  S1  per image: lhsT=x[h,w] (stationary), rhs=EH[h,(kxr|kxi)] -> P1[w,64] psum
  S2  lhsT=FW[:,0:32]/[:,32:64] (ky re/im), rhs=P1s[w, 8img*64] -> A,B[32,512]
  C2  DVE combine -> CB[b][ky32, (kx32,i16,ri2)]
  T1  PE-transpose 128-chunks -> MTall[128=(kxl4,i16,ri2), b*256+(q8,ky32)]
  S3  modal: lhsT=WBD[q,ky][128,128] block-diag complex, rhs=MTall cols (b16)
      -> MO[q][(o16,kxl4,ri2), (ky,b)] -> reorder -> MOs[128, q*512+(b,ky)]
  S4  kx-inverse, accumulate over q: lhsT=IEQr/i[q][8,h'128],
      rhs=MOs[8o:8o+8, q-slice] -> Zr,Zi[h', (b,ky)] -> Zs[o]
  T3  per (o,b): PE-transpose Zs[128,(ri2,ky32)-cols] -> ZT[(ri2,ky32)=64, h']
  S5  lhsT=ZT, rhs=CW[(ri,ky), w'] -> y[h', w'] psum -> out DMA
"""
import os
import sys

import numpy as np

sys.path.insert(0, "/opt/trn_rl_repo")

import concourse.bass as bass  # noqa: E402
import concourse.mybir as mybir  # noqa: E402
from concourse.bass_utils import bass_rust, run_bass_kernel_spmd  # noqa: E402
from concourse.masks import make_identity  # noqa: E402
from concourse.tile import TileContext  # noqa: E402

F32 = mybir.dt.float32
F16 = mybir.dt.float16
NB = 16  # batches per core


def _build_shared_consts():
    h = np.arange(128)
    k32 = np.arange(32)
    ang = 2 * np.pi * np.outer(h, k32) / 128.0
    EH = np.concatenate([np.cos(ang), -np.sin(ang)], axis=1).astype(np.float32)
    FW3 = np.concatenate([np.cos(ang), np.sin(ang), -np.sin(ang)],
                         axis=1).astype(np.float32)
    # IEQR/IEQI [64, 128]: rows (kxl4, ri2, q8) matching the per-o slab
    # fold (8 rows x (q,b,ky) -> 64 rows x (b,ky)); cols h'.
    IEQR = np.zeros((64, 128), np.float32)
    IEQI = np.zeros((64, 128), np.float32)
    for kxl in range(4):
        for q in range(8):
            kx = q * 4 + kxl
            a = 2 * np.pi * h * kx / 128.0
            IEQR[kxl * 16 + 0 * 8 + q] = np.cos(a) / 128.0
            IEQR[kxl * 16 + 1 * 8 + q] = -np.sin(a) / 128.0
            IEQI[kxl * 16 + 0 * 8 + q] = np.sin(a) / 128.0
            IEQI[kxl * 16 + 1 * 8 + q] = np.cos(a) / 128.0
    CW = np.zeros((64, 128), np.float32)
    for ky in range(32):
        c = 1.0 if ky == 0 else 2.0
        a = 2 * np.pi * h * ky / 128.0
        CW[ky] = c * np.cos(a) / 128.0
        CW[32 + ky] = -c * np.sin(a) / 128.0
    CW[32] = 0.0  # irfft ignores Im of bin 0
    CW = np.concatenate([CW, CW], axis=0)  # [128, 128] both bases
    return EH, FW3, IEQR, IEQI, CW


def _build_wbd(w_real_h, w_imag_h):
    """[16i,16o,32kx,32ky] -> WBD [256, 128, 128] block-diag complex tiles."""
    Wr = np.ascontiguousarray(w_real_h.transpose(2, 3, 0, 1)).reshape(8, 4, 32, 16, 16)
    Wi = np.ascontiguousarray(w_imag_h.transpose(2, 3, 0, 1)).reshape(8, 4, 32, 16, 16)
    Wr = Wr.transpose(0, 2, 1, 3, 4)  # [q, ky, kxl, i, o]
    Wi = Wi.transpose(0, 2, 1, 3, 4)
    B7 = np.zeros((8, 32, 4, 16, 2, 16, 2), np.float32)  # [q,ky,kxl,i,ri,o,ri']
    B7[:, :, :, :, 0, :, 0] = Wr
    B7[:, :, :, :, 0, :, 1] = Wi
    B7[:, :, :, :, 1, :, 0] = -Wi
    B7[:, :, :, :, 1, :, 1] = Wr
    full = np.zeros((8, 32, 4, 16, 2, 16, 4, 2), np.float32)
    kk = np.arange(4)
    # rows (kxl,i,ri), cols (o,kxl,ri') -- diagonal in kxl
    full[:, :, kk, :, :, :, kk, :] = B7.transpose(2, 0, 1, 3, 4, 5, 6)
    return np.ascontiguousarray(full.reshape(256, 128, 128))


class _PhaseStop(Exception):
    pass


def _build_graph():
    nc = bass.Bass()
    x_ext = nc.declare_dram_parameter("x", [32, 128, 1024], F16, isOutput=False)
    eh_ext = nc.declare_dram_parameter("eh", [128, 64], F16, isOutput=False)
    fw3_ext = nc.declare_dram_parameter("fw3", [128, 96], F16, isOutput=False)
    wbd_ext = nc.declare_dram_parameter("wbd", [128, 32768], F16, isOutput=False)
    ieqr_ext = nc.declare_dram_parameter("ieqr", [64, 128], F16, isOutput=False)
    ieqi_ext = nc.declare_dram_parameter("ieqi", [64, 128], F16, isOutput=False)
    cw_ext = nc.declare_dram_parameter("cw", [128, 128], F16, isOutput=False)
    y_ext = nc.declare_dram_parameter("out", [16, 4, 128, 512], F16, isOutput=True)

    with TileContext(nc) as tc:
      try:
        with (
            tc.tile_pool(name="consts", bufs=1) as cpool,
            tc.tile_pool(name="slabs", bufs=1) as spool,
            tc.tile_pool(name="work", bufs=6) as wpool,
            tc.tile_pool(name="xts", bufs=16) as xpool,
        ):
            eh_sb = cpool.tile([128, 64], F16, tag="eh")
            nc.sync.dma_start(out=eh_sb, in_=eh_ext[:])
            fw3_sb = cpool.tile([128, 96], F16, tag="fw3")
            nc.sync.dma_start(out=fw3_sb, in_=fw3_ext[:])
            ieqr_sb = cpool.tile([64, 128], F16, tag="ieqr")
            nc.sync.dma_start(out=ieqr_sb, in_=ieqr_ext[:])
            ieqi_sb = cpool.tile([64, 128], F16, tag="ieqi")
            nc.sync.dma_start(out=ieqi_sb, in_=ieqi_ext[:])
            cw_sb = cpool.tile([128, 128], F16, tag="cw")
            nc.sync.dma_start(out=cw_sb, in_=cw_ext[:])
            ident = cpool.tile([128, 128], F16, tag="ident")
            make_identity(nc, ident)

            mtall = spool.tile([128, NB * 256], F16, tag="mtall")
            mosA = spool.tile([64, 4096], F16, tag="mosA")
            mosB = spool.tile([64, 4096], F16, tag="mosB")
            wslab = spool.tile([128, 32768], F16, tag="wslab")
            nc.sync.dma_start(out=wslab, in_=wbd_ext[:])

            # ---------------- Phase F: forward DFTs ----------------
            psF_cm = tc.tile_pool(name="psF", bufs=2, space="PSUM")
            psF = psF_cm.__enter__()
            psF1_cm = tc.tile_pool(name="psF1", bufs=3, space="PSUM")
            psF1 = psF1_cm.__enter__()
            psFt_cm = tc.tile_pool(name="psFt", bufs=1, space="PSUM")
            psFt = psFt_cm.__enter__()
            for b in range(NB):
                cb = wpool.tile([32, 1024], F16, tag="cb")
                for g in range(2):
                    # p1 cols: [img8*kx32 real | img8*kx32 imag]
                    p1 = psF1.tile([128, 512], F32, tag="p1")
                    xt8 = xpool.tile([128, 1024], F16, tag="xt8")
                    nc.sync.dma_start(out=xt8, in_=x_ext[b * 2 + g])
                    for j in range(8):
                        nc.tensor.matmul(p1[:, j * 64:(j + 1) * 64],
                                         lhsT=xt8[:, j * 128:(j + 1) * 128],
                                         rhs=eh_sb, start=True, stop=True)
                    # copy + de-interleave (img,ri,kx) -> (ri,img,kx)
                    p1s = wpool.tile([128, 512], F16, tag="p1s")
                    nc.scalar.copy(
                        out=p1s.rearrange("p (r j k) -> p j r k", r=2, j=8),
                        in_=p1.rearrange("p (j r k) -> p j r k", r=2, j=8))
                    # Xr = cos.P1r + sin.P1i ; Xi = cos.P1i - sin.P1r
                    xr = psF.tile([32, 256], F32, tag="xr")
                    xi = psF.tile([32, 256], F32, tag="xi")
                    nc.tensor.matmul(xr, lhsT=fw3_sb[:, 0:32],
                                     rhs=p1s[:, 0:256], start=True, stop=False)
                    nc.tensor.matmul(xr, lhsT=fw3_sb[:, 32:64],
                                     rhs=p1s[:, 256:512], start=False, stop=True)
                    nc.tensor.matmul(xi, lhsT=fw3_sb[:, 0:32],
                                     rhs=p1s[:, 256:512], start=True, stop=False)
                    nc.tensor.matmul(xi, lhsT=fw3_sb[:, 64:96],
                                     rhs=p1s[:, 0:256], start=False, stop=True)
                    # scatter into CB [32, (kx32, i16, ri2)]
                    cbv = cb.rearrange("p (kx i r) -> p i kx r", i=16, r=2)
                    i0 = g * 8
                    nc.vector.tensor_copy(
                        out=cbv[:, i0:i0 + 8, :, 0],
                        in_=xr.rearrange("p (j k) -> p j k", j=8))
                    nc.vector.tensor_copy(
                        out=cbv[:, i0:i0 + 8, :, 1],
                        in_=xi.rearrange("p (j k) -> p j k", j=8))
                # T1: 8 transposes [32,128] -> [128,32]
                pt = psFt.tile([128, 256], F16, tag="pt")
                for q in range(8):
                    nc.tensor.transpose(pt[:, q * 32:(q + 1) * 32],
                                        cb[:, q * 128:(q + 1) * 128],
                                        ident[0:32, 0:32])
                nc.vector.tensor_copy(out=mtall[:, b * 256:(b + 1) * 256], in_=pt)

            psFt_cm.__exit__(None, None, None)
            psF1_cm.__exit__(None, None, None)
            psF_cm.__exit__(None, None, None)

            if os.environ.get("KPHASES", "FMI") == "F":
                raise _PhaseStop()
            # ---------------- Phase M: modal contraction ----------------
            mtv = mtall.rearrange("p (b u) -> p u b", u=256)  # [128, 256, 16]
            wsv = wslab.rearrange("r (u c) -> r u c", c=128)
            with tc.tile_pool(name="psM", bufs=2, space="PSUM") as psM:
                for q in range(8):
                    mo = psM.tile([128, 512], F32, tag="mo")
                    for ky in range(32):
                        u = q * 32 + ky
                        nc.tensor.matmul(mo[:, ky * 16:ky * 16 + NB],
                                         lhsT=wsv[:, u, :],
                                         rhs=mtv[:, u, :], start=True, stop=True)
                    for rb, mosx in ((0, mosA), (64, mosB)):
                        dst = mosx[:, q * 512:(q + 1) * 512].rearrange(
                            "p (b k) -> p k b", b=16)
                        srcv = mo[rb:rb + 64, :].rearrange(
                            "p (k b) -> p k b", b=16)
                        if rb == 0:
                            nc.vector.tensor_copy(out=dst, in_=srcv)
                        else:
                            nc.scalar.copy(out=dst, in_=srcv)

            if os.environ.get("KPHASES", "FMI") == "FM":
                raise _PhaseStop()
            # ---------------- Phase I: inverse DFTs ----------------
            # matmul operands need partition base in {0,32,64}; DMA the 8-row
            # o-slices of mos down to base-0 tiles, 4 o's (one out-group) at
            # a time to bound SBUF.
            with (
                tc.tile_pool(name="zsp", bufs=2) as zspool,
                tc.tile_pool(name="slb", bufs=8) as slbpool,
                tc.tile_pool(name="psI", bufs=2, space="PSUM") as psI,
            ):
                for og in range(4):
                    zsg = zspool.tile([128, 4096], F16, tag="zs")
                    mosx = mosA if og < 2 else mosB
                    for ol in range(4):
                        o = og * 4 + ol
                        op = o % 8
                        # Fold [8, (q8,b,ky)] -> [64=(kxl,ri,q), (b,ky)]:
                        # same flat element order, repartitioned by DMA.
                        slab = slbpool.tile([64, 512], F16, tag="slab")
                        nc.sync.dma_start(
                            out=slab.rearrange("p (b k) -> p b k", b=16),
                            in_=mosx[op * 8:op * 8 + 8, :].rearrange(
                                "p (q b k) -> p q b k", q=8, b=16))
                        zr = psI.tile([128, 512], F32, tag="zr")
                        zi = psI.tile([128, 512], F32, tag="zi")
                        nc.tensor.matmul(zr, lhsT=ieqr_sb, rhs=slab,
                                         start=True, stop=True)
                        nc.tensor.matmul(zi, lhsT=ieqi_sb, rhs=slab,
                                         start=True, stop=True)
                        # zsg free layout: (o4, b16, ri2, ky32)
                        zsgv = zsg.rearrange("p (o b r k) -> p o b r k",
                                             o=4, b=16, r=2)
                        nc.scalar.copy(out=zsgv[:, ol, :, 0, :],
                                       in_=zr.rearrange("p (b k) -> p b k", b=16))
                        nc.scalar.copy(out=zsgv[:, ol, :, 1, :],
                                       in_=zi.rearrange("p (b k) -> p b k", b=16))
                    for b2 in range(NB // 2):
                        b0 = b2 * 2
                        zt_ps = psI.tile([128, 512], F16, tag="zt")
                        for ol in range(4):
                            # [128, (b2,(ri,ky))=128] -> [128=(b2,ri,ky), h']
                            nc.tensor.transpose(
                                zt_ps[:, ol * 128:(ol + 1) * 128],
                                zsg[:, ol * 1024 + b0 * 64:
                                    ol * 1024 + b0 * 64 + 128],
                                ident)
                        zt_sb = wpool.tile([128, 512], F16, tag="ztsb")
                        nc.vector.tensor_copy(out=zt_sb, in_=zt_ps)
                        for db in range(2):
                            b = b0 + db
                            rb = db * 64
                            # lhsT = CW (stationary const) -> y transposed
                            # [w', (ol, h')]; host unpermutes.
                            y_ps = psI.tile([128, 512], F32, tag="y")
                            for ol in range(4):
                                nc.tensor.matmul(
                                    y_ps[:, ol * 128:(ol + 1) * 128],
                                    lhsT=cw_sb[rb:rb + 64, :],
                                    rhs=zt_sb[rb:rb + 64,
                                              ol * 128:(ol + 1) * 128],
                                    start=True, stop=True)
                            out_sb = wpool.tile([128, 512], F16, tag="outsb")
                            if db == 0:
                                nc.scalar.copy(out=out_sb, in_=y_ps)
                            else:
                                nc.vector.tensor_copy(out=out_sb, in_=y_ps)
                            nc.sync.dma_start(out=y_ext[b, og], in_=out_sb)
      except _PhaseStop:
        pass
    # TRN2 instructions carry limited sync-wait slots; split excess waits
    # (same passes Bacc.compile runs).
    bass_rust.move_matmul_waits_to_ldweights(nc.m)
    bass_rust.generate_event_semaphores(nc)
    return nc


_CACHE = {}


def kernel(x, w_real, w_imag, bias):
    x = np.asarray(x, np.float32)
    w_real = np.asarray(w_real, np.float32)
    w_imag = np.asarray(w_imag, np.float32)
    bias = np.asarray(bias, np.float32)

    if "nc" not in _CACHE:
        _CACHE["nc"] = _build_graph()
        _CACHE["consts"] = _build_shared_consts()
    nc = _CACHE["nc"]
    EH, FW3, IEQR, IEQI, CW = _CACHE["consts"]

    in_maps = []
    for c in range(8):
        head, half = c // 2, c % 2
        xs = x[half * 16:(half + 1) * 16, head * 16:(head + 1) * 16]
        # [16b,16i,128,128] -> [(b,g), h, (i8, w)] image-interleaved
        xs = np.ascontiguousarray(
            xs.reshape(16, 2, 8, 128, 128).transpose(0, 1, 3, 2, 4)
        ).reshape(32, 128, 1024).astype(np.float16)
        key = ("wbd", head)
        if key not in _CACHE:
            wb = _build_wbd(w_real[head], w_imag[head])  # [256u,128r,128c]
            _CACHE[key] = np.ascontiguousarray(
                wb.transpose(1, 0, 2)).reshape(128, 32768).astype(np.float16)
        in_maps.append({"x": xs, "eh": EH.astype(np.float16),
                        "fw3": FW3.astype(np.float16),
                        "wbd": _CACHE[key], "ieqr": IEQR.astype(np.float16),
                        "ieqi": IEQI.astype(np.float16),
                        "cw": CW.astype(np.float16)})

    trace = os.environ.get("KERNEL_TRACE", "0") == "1"
    res = run_bass_kernel_spmd(nc, in_maps, core_ids=list(range(8)), trace=trace)
    _CACHE["exec_time_ns"] = res.exec_time_ns

    out = np.empty((32, 64, 128, 128), np.float32)
    for c in range(8):
        head, half = c // 2, c % 2
        # [b, og, w', (ol, h')] -> [b, o, h, w]
        ys = res.results[c]["out"].astype(np.float32).reshape(
            16, 4, 128, 4, 128).transpose(0, 1, 3, 4, 2).reshape(
            16, 16, 128, 128)
        out[half * 16:(half + 1) * 16, head * 16:(head + 1) * 16] = ys
    return out + bias[None]
